# revision 20
# baseline (speedup 1.0000x reference)
"""Trainium2 Bass kernel for nn_DecoderLayer (self-attn + cross-attn + FFN).

Sharding: 8 cores = 2 batches x 4 query-blocks of 512 tokens (data/sequence
parallel, zero collectives). Each core recomputes the full K/V for its batch
and runs its 512 queries through the whole layer.

Layouts (per core):
  activations transposed [D, tok]; scores computed transposed [k, q] so the
  softmax denominator comes free via a ones-column appended to V; LayerNorm
  over the partition dim via ones-vector matmuls + PE broadcast.
Matmuls run in float32r (1 cyc/row vs 4 for fp32) unless DTYPE_MODE='f32'.
"""

import sys

if "/opt/trn_rl_repo" not in sys.path:
    sys.path.insert(0, "/opt/trn_rl_repo")

import numpy as np

D = 1024
S = 2048
QTOK = 512          # queries per core
H = 16
HD = 64
FFN = 4096
EPS = 1e-5
NCORES = 8
NG = 4              # head groups per attention
GH = 4              # heads per group
GD = GH * HD        # 256 dout per group
NBLK = 8            # token blocks for K/V projection streaming
TB = S // NBLK      # 256
VROW = GH * (HD + 1) + 1   # 261 cols per k-tile in V'' (ones interleaved)

DTYPE_MODE = "f32r"  # 'f32r' | 'f32'

_PROGRAM_CACHE = {}


def _build_program(mode=DTYPE_MODE):
    import contextlib

    import concourse.bacc as bacc
    import concourse.tile as tile
    from concourse import mybir

    f32 = mybir.dt.float32
    f32r = mybir.dt.float32r
    AF = mybir.ActivationFunctionType
    Alu = mybir.AluOpType

    def mm(ap):
        """cast a matmul operand to the fast dtype"""
        return ap.bitcast(f32r) if mode == "f32r" else ap

    rc = mm  # producers of matmul-consumed data must emit f32r-rounded output

    nc = bacc.Bacc("TRN2", target_bir_lowering=False)

    # ---- DRAM parameters (per-core data supplied via in_maps) ----
    def din(name, shape):
        return nc.declare_dram_parameter(name, list(shape), f32, isOutput=False)

    xT = din("xT", (D, S))            # hidden[b].T
    xqT = din("xqT", (D, QTOK))       # hidden[b, q0:q0+512].T
    encT = din("encT", (D, S))        # encoder[b].T
    w = {}
    for p in ("sa", "ca"):
        for wn in ("wq", "wk", "wv"):
            w[f"{p}_{wn}"] = din(f"{p}_{wn}", (D, D))
        for bn in ("bq", "bk"):
            w[f"{p}_{bn}"] = din(f"{p}_{bn}", (128, 8))   # reshaped (8,128).T
        w[f"{p}_bv"] = din(f"{p}_bv", (128, D))           # row-broadcast
    fc1_w = din("fc1_w", (D, FFN))
    fc2_w = din("fc2_w", (FFN, D))
    fc1_b = din("fc1_b", (128, 32))
    fc2_b = din("fc2_b", (128, 8))
    for i in (1, 2, 3):
        w[f"ln{i}_g"] = din(f"ln{i}_g", (128, 8))
        w[f"ln{i}_b"] = din(f"ln{i}_b", (128, 8))
    outT = nc.declare_dram_parameter("outT", [D, QTOK], f32, isOutput=True)

    with tile.TileContext(nc) as tc:
        with contextlib.ExitStack() as ctx:
            consts = ctx.enter_context(tc.tile_pool(name="consts", bufs=1))
            pkt = ctx.enter_context(tc.tile_pool(name="pkt", bufs=1))
            pvp = ctx.enter_context(tc.tile_pool(name="pvp", bufs=1))
            pqt = ctx.enter_context(tc.tile_pool(name="pqt", bufs=1))
            pxa = ctx.enter_context(tc.tile_pool(name="pxa", bufs=2))
            pxblk = ctx.enter_context(tc.tile_pool(name="pxblk", bufs=2))
            pwatt = ctx.enter_context(tc.tile_pool(name="pwatt", bufs=1))
            pwslab = ctx.enter_context(tc.tile_pool(name="pwslab", bufs=3))
            pctx = ctx.enter_context(tc.tile_pool(name="pctx", bufs=1))
            pexp = ctx.enter_context(tc.tile_pool(name="pexp", bufs=3))
            psq = ctx.enter_context(tc.tile_pool(name="psq", bufs=2))
            pstat = ctx.enter_context(tc.tile_pool(name="pstat", bufs=1))
            ppp = ctx.enter_context(tc.tile_pool(name="ppp", bufs=2, space="PSUM"))
            pps = ctx.enter_context(tc.tile_pool(name="pps", bufs=2, space="PSUM"))
            ppc = ctx.enter_context(tc.tile_pool(name="ppc", bufs=2, space="PSUM"))

            # ---- constants ----
            ones = consts.tile([128, 128], f32, tag="ones")
            nc.vector.memset(ones, 1.0)
            xq_sb = consts.tile([128, 8, QTOK], f32, tag="xq")
            xqT_v = xqT.rearrange("(c p) t -> p c t", p=128)
            for _c in range(8):
                nc.sync.dma_start(
                    out=rc(xq_sb[:, _c, :]), in_=rc(xqT_v[:, _c, :])
                )
            sb = {}
            for name, hnd in w.items():
                if name.endswith(("wq", "wk", "wv")):
                    continue
                sb[name] = consts.tile([128, hnd.shape[1]], f32, tag=name, name=name)
                nc.sync.dma_start(out=sb[name], in_=hnd[:, :])
            fc1b_sb = consts.tile([128, 32], f32, tag="fc1b")
            nc.sync.dma_start(out=fc1b_sb, in_=fc1_b[:, :])
            fc2b_sb = consts.tile([128, 8], f32, tag="fc2b")
            nc.sync.dma_start(out=fc2b_sb, in_=fc2_b[:, :])

            def attention(pfx, src_dram, srcq_sb, ctx_tile):
                """One MHA: Q from srcq_sb [128,8,512], K/V from src_dram [D,S].
                Writes normalized ctx^T into ctx_tile [128,8,512]."""
                wq_h, wk_h, wv_h = w[f"{pfx}_wq"], w[f"{pfx}_wk"], w[f"{pfx}_wv"]
                bq_sb, bk_sb, bv_sb = sb[f"{pfx}_bq"], sb[f"{pfx}_bk"], sb[f"{pfx}_bv"]
                for g in range(NG):
                    # -- Q projection for this group (2 dtiles of 128) --
                    qt = pqt.tile([128, 2, QTOK], f32, tag="qt", name="qt")
                    qacc = [ppp.tile([128, QTOK], f32, tag="pp", name=f"qacc{_i}")
                            for _i in range(2)]
                    for c in range(8):
                        qs = pwslab.tile([128, GD], f32, tag="wslab", name="qs")
                        nc.sync.dma_start(
                            out=rc(qs),
                            in_=rc(wq_h[c * 128 : (c + 1) * 128,
                                        g * GD : (g + 1) * GD]),
                        )
                        for i in range(2):
                            nc.tensor.matmul(
                                qacc[i][:, :],
                                mm(qs[:, i * 128 : (i + 1) * 128]),
                                mm(srcq_sb[:, c, :]),
                                start=(c == 0),
                                stop=(c == 7),
                            )
                    for i in range(2):
                        nc.vector.tensor_scalar_add(
                            rc(qt[:, i, :]), qacc[i][:, :],
                            bq_sb[:, g * 2 + i : g * 2 + i + 1],
                        )
                    # -- K / V projections (streamed over token blocks) --
                    wkg = pwatt.tile([128, 8, GD], f32, tag="wkh", name="wkg")
                    nc.sync.dma_start(
                        out=rc(wkg),
                        in_=rc(wk_h[:, g * GD : (g + 1) * GD].rearrange(
                            "(c p) n -> p c n", p=128
                        )),
                    )
                    wvg = pwatt.tile([128, 8, GD], f32, tag="wvh", name="wvg")
                    nc.sync.dma_start(
                        out=rc(wvg),
                        in_=rc(wv_h[:, g * GD : (g + 1) * GD].rearrange(
                            "(c p) n -> p c n", p=128
                        )),
                    )
                    kt = pkt.tile([128, 2, S], f32, tag="kt", name="kt")
                    vp = pvp.tile([128, 16, VROW], f32, tag="vp", name="vp")
                    # ones columns at h*65 for h=0..3, plus trailing col 260
                    nc.vector.memset(
                        vp[:, :, 0 : 65 * GH].rearrange(
                            "p t (h u) -> p t h u", u=65
                        )[:, :, :, 0:1],
                        1.0,
                    )
                    nc.vector.memset(vp[:, :, VROW - 1 : VROW], 1.0)
                    for blk in range(NBLK):
                        xb = pxblk.tile([128, 8, TB], f32, tag="xblk", name="xb")
                        nc.sync.dma_start(
                            out=rc(xb),
                            in_=rc(src_dram[:, blk * TB : (blk + 1) * TB]
                                   .rearrange("(c p) t -> p c t", p=128)),
                        )
                        for dt in range(2):
                            kacc = ppp.tile([128, TB], f32, tag="pp",
                                            name="kacc")
                            for c in range(8):
                                nc.tensor.matmul(
                                    kacc[:, :],
                                    mm(wkg[:, c, dt * 128 : (dt + 1) * 128]),
                                    mm(xb[:, c, :]),
                                    start=(c == 0),
                                    stop=(c == 7),
                                )
                            nc.vector.tensor_scalar_add(
                                rc(kt[:, dt, blk * TB : (blk + 1) * TB]),
                                kacc[:, :],
                                bk_sb[:, g * 2 + dt : g * 2 + dt + 1],
                            )
                        for tt in range(TB // 128):
                            vacc = ppp.tile([128, GD], f32, tag="pp",
                                            name="vacc")
                            for c in range(8):
                                nc.tensor.matmul(
                                    vacc[:, :],
                                    mm(xb[:, c, tt * 128 : (tt + 1) * 128]),
                                    mm(wvg[:, c, :]),
                                    start=(c == 0),
                                    stop=(c == 7),
                                )
                            j = blk * (TB // 128) + tt
                            dst = vp[:, j, 1 : 1 + 65 * GH].rearrange(
                                "p (h u) -> p h u", u=65
                            )[:, :, 0:HD]
                            nc.vector.tensor_tensor(
                                out=rc(dst),
                                in0=vacc.rearrange("p (h u) -> p h u", u=HD),
                                in1=bv_sb[:, g * GD : (g + 1) * GD].rearrange(
                                    "p (h u) -> p h u", u=HD
                                ),
                                op=Alu.add,
                            )
                    # -- attention for the 4 heads of this group --
                    for h in range(GH):
                        ha = g * GH + h
                        par = h % 2
                        dt = h // 2
                        pc = ppc.tile([128, QTOK], f32, tag="pc", name="pc")
                        wm = ppp.tile([128, QTOK], f32, tag="pp", name="wm")
                        ndum = 0
                        vlo = h * 65 + 1   # [v0..v63, one] -> denom row 64
                        for jg in range(8):
                            ps = pps.tile([128, 2, QTOK], f32, tag="ps",
                                          name="ps")
                            for js in range(2):
                                j = jg * 2 + js
                                nc.tensor.matmul(
                                    ps[:, js, :],
                                    mm(kt[par * 64 : par * 64 + 64, dt,
                                          j * 128 : (j + 1) * 128]),
                                    mm(qt[par * 64 : par * 64 + 64, dt, :]),
                                    start=True,
                                    stop=True,
                                )
                            et = pexp.tile([128, 2, QTOK], f32, tag="e",
                                           name="et")
                            nc.scalar.activation(
                                out=rc(et), in_=ps, func=AF.Exp,
                                scale=1.0 / (HD ** 0.5),
                            )
                            # keepalive: one accumulation group per head of
                            # dependency-free matmuls, slotted where the PE
                            # would otherwise wait on exp, so the HAM clock
                            # gate sees continuous PE activity (12 per head)
                            for _d in range(2 if jg % 2 == 0 else 1):
                                nc.tensor.matmul(
                                    wm[:, :],
                                    mm(ones[:, 0:128]),
                                    mm(xq_sb[:, 0, :]),
                                    start=(ndum == 0),
                                    stop=(ndum == 11),
                                )
                                ndum += 1
                            for js in range(2):
                                j = jg * 2 + js
                                nc.tensor.matmul(
                                    pc[0:65, :],
                                    mm(vp[:, j, vlo : vlo + 65]),
                                    mm(et[:, js, :]),
                                    start=(jg == 0 and js == 0),
                                    stop=(jg == 7 and js == 1),
                                )
                        # normalize by the denominator row (psum row 64)
                        rd = psq.tile([128, QTOK], f32, tag="sq", name="rd")
                        with nc.allow_low_precision("f32r bcast feed"):
                            nc.vector.reciprocal(
                                out=rc(rd[64:65, :]), in_=pc[64:65, :]
                            )
                        rb = pps.tile([128, 2, QTOK], f32, tag="ps", name="rb")
                        nc.tensor.matmul(
                            rb[0:64, 0, :],
                            mm(ones[64:65, 0:64]),
                            mm(rd[64:65, :]),
                            start=True,
                            stop=True,
                        )
                        cu = pexp.tile([128, QTOK], f32, tag="cu", name="cu")
                        nc.vector.tensor_copy(out=cu[0:64, :], in_=pc[0:64, :])
                        if par == 0:
                            nc.vector.tensor_tensor(
                                out=ctx_tile[0:64, ha // 2, :],
                                in0=cu[0:64, :],
                                in1=rb[0:64, 0, :],
                                op=Alu.mult,
                            )
                        else:
                            # engines can't cross partitions: normalize at
                            # base 0, then DMA-shift into partitions 64+
                            cn = psq.tile([128, QTOK], f32, tag="sq",
                                          name="cn")
                            nc.vector.tensor_tensor(
                                out=cn[0:64, :],
                                in0=cu[0:64, :],
                                in1=rb[0:64, 0, :],
                                op=Alu.mult,
                            )
                            nc.sync.dma_start(
                                out=ctx_tile[64:128, ha // 2, :],
                                in_=cn[0:64, :],
                            )

            def layernorm(x1, gname, out_tile, dma_out=None):
                """out = LN(x1) * g + b, normalizing over the partition dim."""
                g_sb, b_sb = sb[f"{gname}_g"], sb[f"{gname}_b"]
                sum_ps = ppp.tile([1, QTOK], f32, tag="pp")
                sq_ps = ppp.tile([1, QTOK], f32, tag="pp")
                for i in range(8):
                    nc.tensor.matmul(
                        sum_ps[:, :], mm(ones[:, 0:1]), mm(x1[:, i, :]),
                        start=(i == 0), stop=(i == 7),
                    )
                for i in range(8):
                    sqt = psq.tile([128, QTOK], f32, tag="sq")
                    nc.vector.tensor_mul(rc(sqt), x1[:, i, :], x1[:, i, :])
                    nc.tensor.matmul(
                        sq_ps[:, :], mm(ones[:, 0:1]), mm(sqt[:, :]),
                        start=(i == 0), stop=(i == 7),
                    )
                s_sb = pstat.tile([1, 2, QTOK], f32, tag="s_sb")
                nc.vector.tensor_copy(out=rc(s_sb[0:1, 0, :]), in_=sum_ps[:, :])
                nc.vector.tensor_copy(out=rc(s_sb[0:1, 1, :]), in_=sq_ps[:, :])
                bps = pps.tile([128, 2, QTOK], f32, tag="ps")
                for k in range(2):
                    nc.tensor.matmul(
                        bps[:, k, :], mm(ones[0:1, 0:128]), mm(s_sb[0:1, k, :]),
                        start=True, stop=True,
                    )
                meanb = pstat.tile([128, QTOK], f32, tag="meanb")
                nc.vector.tensor_scalar_mul(meanb, bps[:, 0, :], 1.0 / D)
                varb = pstat.tile([128, QTOK], f32, tag="varb")
                nc.vector.tensor_scalar_mul(varb, bps[:, 1, :], 1.0 / D)
                tmpb = pstat.tile([128, QTOK], f32, tag="tmpb")
                nc.vector.tensor_mul(tmpb, meanb, meanb)
                nc.vector.tensor_sub(varb, varb, tmpb)
                nc.vector.tensor_scalar_add(varb, varb, EPS)
                nc.vector.reciprocal(out=tmpb, in_=varb)
                rstdb = pstat.tile([128, QTOK], f32, tag="rstdb")
                nc.scalar.activation(out=rstdb, in_=tmpb, func=AF.Sqrt)
                for i in range(8):
                    t1 = psq.tile([128, QTOK], f32, tag="sq")
                    nc.vector.tensor_sub(t1, x1[:, i, :], meanb)
                    nc.vector.tensor_mul(t1, t1, rstdb)
                    nc.vector.tensor_scalar(
                        out=rc(out_tile[:, i, :]), in0=t1,
                        scalar1=g_sb[:, i : i + 1], scalar2=b_sb[:, i : i + 1],
                        op0=Alu.mult, op1=Alu.add,
                    )
                    if dma_out is not None:
                        nc.sync.dma_start(
                            out=dma_out[:, i, :], in_=out_tile[:, i, :]
                        )

            def eight_psums():
                """8 one-bank [128,512] accumulators spanning all three pools."""
                a = [ppp.tile([128, QTOK], f32, tag="pp", name=f"fa{_i}") for _i in range(2)]
                b = [ppc.tile([128, QTOK], f32, tag="pc", name=f"fb{_i}") for _i in range(2)]
                c_ = [pps.tile([128, 2, QTOK], f32, tag="ps", name=f"fc{_i}") for _i in range(2)]
                return [a[0][:, :], a[1][:, :], b[0][:, :], b[1][:, :],
                        c_[0][:, 0, :], c_[0][:, 1, :], c_[1][:, 0, :], c_[1][:, 1, :]]

            # ================= self-attention =================
            ctx1 = pctx.tile([128, 8, QTOK], f32, tag="ctx")
            attention("sa", xT, xq_sb, ctx1)
            x1 = pxa.tile([128, 8, QTOK], f32, tag="xa")
            nc.vector.tensor_add(rc(x1), xq_sb, ctx1)
            x2 = pxa.tile([128, 8, QTOK], f32, tag="xa")
            layernorm(x1, "ln1", x2)

            # ================= cross-attention =================
            ctx2 = pctx.tile([128, 8, QTOK], f32, tag="ctx")
            attention("ca", encT, x2, ctx2)
            x3p = pxa.tile([128, 8, QTOK], f32, tag="xa")
            nc.vector.tensor_add(rc(x3p), x2, ctx2)
            x3 = pqt.tile([128, 8, QTOK], f32, tag="qt")
            layernorm(x3p, "ln2", x3)

            # ================= feed-forward =================
            h2acc = pctx.tile([128, 8, QTOK], f32, tag="ctx")
            for qtr in range(4):
                h1q = pkt.tile([128, 8, QTOK], f32, tag="kt")
                h1accs = eight_psums()
                for c in range(8):
                    f1s = pwslab.tile([128, 1024], f32, tag="wslab")
                    nc.sync.dma_start(
                        out=rc(f1s),
                        in_=rc(fc1_w[c * 128 : (c + 1) * 128,
                                     qtr * 1024 : (qtr + 1) * 1024]),
                    )
                    for f in range(8):
                        nc.tensor.matmul(
                            h1accs[f],
                            mm(f1s[:, f * 128 : (f + 1) * 128]),
                            mm(x3[:, c, :]),
                            start=(c == 0),
                            stop=(c == 7),
                        )
                for f in range(8):
                    nc.scalar.activation(
                        out=rc(h1q[:, f, :]), in_=h1accs[f], func=AF.Relu,
                        bias=fc1b_sb[:, qtr * 8 + f : qtr * 8 + f + 1],
                    )
                h2accs = eight_psums()
                for f in range(8):
                    f2s = pwslab.tile([128, 1024], f32, tag="wslab")
                    nc.sync.dma_start(
                        out=rc(f2s),
                        in_=rc(fc2_w[qtr * 1024 + f * 128
                                     : qtr * 1024 + (f + 1) * 128, :]),
                    )
                    for i in range(8):
                        nc.tensor.matmul(
                            h2accs[i],
                            mm(f2s[:, i * 128 : (i + 1) * 128]),
                            mm(h1q[:, f, :]),
                            start=(f == 0),
                            stop=(f == 7),
                        )
                for i in range(8):
                    if qtr == 0:
                        nc.vector.tensor_copy(out=h2acc[:, i, :], in_=h2accs[i])
                    else:
                        nc.vector.tensor_tensor(
                            out=h2acc[:, i, :], in0=h2acc[:, i, :],
                            in1=h2accs[i], op=Alu.add,
                        )
            x4p = pxa.tile([128, 8, QTOK], f32, tag="xa")
            for i in range(8):
                nc.vector.tensor_scalar_add(
                    h2acc[:, i, :], h2acc[:, i, :], fc2b_sb[:, i : i + 1]
                )
            nc.vector.tensor_add(rc(x4p), x3, h2acc)
            x4 = pxa.tile([128, 8, QTOK], f32, tag="xa")
            layernorm(x4p, "ln3", x4,
                      dma_out=outT.rearrange("(i p) t -> p i t", p=128))

    nc.compile()
    return nc


def _get_program(mode=DTYPE_MODE):
    if mode not in _PROGRAM_CACHE:
        _PROGRAM_CACHE[mode] = _build_program(mode)
    return _PROGRAM_CACHE[mode]


def _make_in_maps(inputs):
    f = np.float32

    def colmajor8(v):  # [1024] -> [128, 8] with [p, i] = v[i*128+p]
        return np.ascontiguousarray(v.reshape(8, 128).T.astype(f))

    shared = {}
    for p in ("sa", "ca"):
        for wn in ("wq", "wk", "wv"):
            shared[f"{p}_{wn}"] = np.ascontiguousarray(inputs[f"{p}_{wn}"], dtype=f)
        for bn in ("bq", "bk"):
            shared[f"{p}_{bn}"] = colmajor8(np.asarray(inputs[f"{p}_{bn}"]))
        shared[f"{p}_bv"] = np.ascontiguousarray(
            np.broadcast_to(np.asarray(inputs[f"{p}_bv"], dtype=f), (128, D))
        )
    shared["fc1_w"] = np.ascontiguousarray(inputs["fc1_w"], dtype=f)
    shared["fc2_w"] = np.ascontiguousarray(inputs["fc2_w"], dtype=f)
    shared["fc1_b"] = np.ascontiguousarray(
        np.asarray(inputs["fc1_b"]).reshape(32, 128).T.astype(f)
    )
    shared["fc2_b"] = colmajor8(np.asarray(inputs["fc2_b"]))
    for i in (1, 2, 3):
        shared[f"ln{i}_g"] = colmajor8(np.asarray(inputs[f"ln{i}_g"]))
        shared[f"ln{i}_b"] = colmajor8(np.asarray(inputs[f"ln{i}_b"]))

    hs = np.asarray(inputs["hidden_states"], dtype=f)
    enc = np.asarray(inputs["encoder_hidden_states"], dtype=f)
    in_maps = []
    for c in range(NCORES):
        b, q0 = c // 4, (c % 4) * QTOK
        m = dict(shared)
        m["xT"] = np.ascontiguousarray(hs[b].T)
        m["xqT"] = np.ascontiguousarray(hs[b, q0 : q0 + QTOK, :].T)
        m["encT"] = np.ascontiguousarray(enc[b].T)
        in_maps.append(m)
    return in_maps


def kernel(**inputs):
    from concourse.bass_utils import run_bass_kernel_spmd

    nc = _get_program()
    in_maps = _make_in_maps(inputs)
    res = run_bass_kernel_spmd(nc, in_maps, core_ids=list(range(NCORES)))
    out = np.empty((2, S, D), np.float32)
    for c in range(NCORES):
        b, q0 = c // 4, (c % 4) * QTOK
        out[b, q0 : q0 + QTOK, :] = res.results[c]["outT"].T
    return out


# revision 21
# speedup vs baseline: 1.0970x; 1.0970x over previous
"""Trainium2 Bass kernel for nn_DecoderLayer (self-attn + cross-attn + FFN).

Sharding: 8 cores = 2 batches x 4 query-blocks of 512 tokens (data/sequence
parallel, zero collectives). Each core recomputes the full K/V for its batch
and runs its 512 queries through the whole layer.

Layouts (per core):
  activations transposed [D, tok]; scores computed transposed [k, q] so the
  softmax denominator comes free via a ones-column appended to V; LayerNorm
  over the partition dim via ones-vector matmuls + PE broadcast.
Matmuls run in float32r (1 cyc/row vs 4 for fp32) unless DTYPE_MODE='f32'.
"""

import sys

if "/opt/trn_rl_repo" not in sys.path:
    sys.path.insert(0, "/opt/trn_rl_repo")

import numpy as np

D = 1024
S = 2048
QTOK = 512          # queries per core
H = 16
HD = 64
FFN = 4096
EPS = 1e-5
NCORES = 8
NG = 4              # head groups per attention
GH = 4              # heads per group
GD = GH * HD        # 256 dout per group
NBLK = 8            # token blocks for K/V projection streaming
TB = S // NBLK      # 256
VROW = GH * (HD + 1) + 1   # 261 cols per k-tile in V'' (ones interleaved)

DTYPE_MODE = "f32r"  # 'f32r' | 'f32'

_PROGRAM_CACHE = {}


def _build_program(mode=DTYPE_MODE):
    import contextlib

    import concourse.bacc as bacc
    import concourse.tile as tile
    from concourse import mybir

    f32 = mybir.dt.float32
    f32r = mybir.dt.float32r
    AF = mybir.ActivationFunctionType
    Alu = mybir.AluOpType

    def mm(ap):
        """cast a matmul operand to the fast dtype"""
        return ap.bitcast(f32r) if mode == "f32r" else ap

    rc = mm  # producers of matmul-consumed data must emit f32r-rounded output

    nc = bacc.Bacc("TRN2", target_bir_lowering=False)

    # ---- DRAM parameters (per-core data supplied via in_maps) ----
    def din(name, shape):
        return nc.declare_dram_parameter(name, list(shape), f32, isOutput=False)

    xT = din("xT", (D, S))            # hidden[b].T
    xqT = din("xqT", (D, QTOK))       # hidden[b, q0:q0+512].T
    encT = din("encT", (D, S))        # encoder[b].T
    w = {}
    for p in ("sa", "ca"):
        for wn in ("wq", "wk", "wv"):
            w[f"{p}_{wn}"] = din(f"{p}_{wn}", (D, D))
        for bn in ("bq", "bk"):
            w[f"{p}_{bn}"] = din(f"{p}_{bn}", (128, 8))   # reshaped (8,128).T
        w[f"{p}_bv"] = din(f"{p}_bv", (128, D))           # row-broadcast
    fc1_w = din("fc1_w", (D, FFN))
    fc2_w = din("fc2_w", (FFN, D))
    fc1_b = din("fc1_b", (128, 32))
    fc2_b = din("fc2_b", (128, 8))
    for i in (1, 2, 3):
        w[f"ln{i}_g"] = din(f"ln{i}_g", (128, 8))
        w[f"ln{i}_b"] = din(f"ln{i}_b", (128, 8))
    outT = nc.declare_dram_parameter("outT", [D, QTOK], f32, isOutput=True)

    with tile.TileContext(nc) as tc:
        with contextlib.ExitStack() as ctx:
            consts = ctx.enter_context(tc.tile_pool(name="consts", bufs=1))
            pkt = ctx.enter_context(tc.tile_pool(name="pkt", bufs=1))
            pvp = ctx.enter_context(tc.tile_pool(name="pvp", bufs=1))
            pqt = ctx.enter_context(tc.tile_pool(name="pqt", bufs=1))
            pxa = ctx.enter_context(tc.tile_pool(name="pxa", bufs=2))
            pxblk = ctx.enter_context(tc.tile_pool(name="pxblk", bufs=2))
            pwatt = ctx.enter_context(tc.tile_pool(name="pwatt", bufs=1))
            pwslab = ctx.enter_context(tc.tile_pool(name="pwslab", bufs=3))
            pctx = ctx.enter_context(tc.tile_pool(name="pctx", bufs=1))
            pexp = ctx.enter_context(tc.tile_pool(name="pexp", bufs=3))
            psq = ctx.enter_context(tc.tile_pool(name="psq", bufs=2))
            pstat = ctx.enter_context(tc.tile_pool(name="pstat", bufs=1))
            ppp = ctx.enter_context(tc.tile_pool(name="ppp", bufs=2, space="PSUM"))
            pps = ctx.enter_context(tc.tile_pool(name="pps", bufs=2, space="PSUM"))
            ppc = ctx.enter_context(tc.tile_pool(name="ppc", bufs=2, space="PSUM"))

            # ---- constants ----
            ones = consts.tile([128, 128], f32, tag="ones")
            nc.vector.memset(ones, 1.0)
            xq_sb = consts.tile([128, 8, QTOK], f32, tag="xq")
            xqT_v = xqT.rearrange("(c p) t -> p c t", p=128)
            for _c in range(8):
                nc.sync.dma_start(
                    out=rc(xq_sb[:, _c, :]), in_=rc(xqT_v[:, _c, :])
                )
            sb = {}
            for name, hnd in w.items():
                if name.endswith(("wq", "wk", "wv")):
                    continue
                sb[name] = consts.tile([128, hnd.shape[1]], f32, tag=name, name=name)
                nc.sync.dma_start(out=sb[name], in_=hnd[:, :])
            fc1b_sb = consts.tile([128, 32], f32, tag="fc1b")
            nc.sync.dma_start(out=fc1b_sb, in_=fc1_b[:, :])
            fc2b_sb = consts.tile([128, 8], f32, tag="fc2b")
            nc.sync.dma_start(out=fc2b_sb, in_=fc2_b[:, :])

            def attention(pfx, src_dram, srcq_sb, ctx_tile):
                """One MHA: Q from srcq_sb [128,8,512], K/V from src_dram [D,S].
                Writes normalized ctx^T into ctx_tile [128,8,512]."""
                wq_h, wk_h, wv_h = w[f"{pfx}_wq"], w[f"{pfx}_wk"], w[f"{pfx}_wv"]
                bq_sb, bk_sb, bv_sb = sb[f"{pfx}_bq"], sb[f"{pfx}_bk"], sb[f"{pfx}_bv"]
                for g in range(NG):
                    # -- Q projection for this group (2 dtiles of 128) --
                    qt = pqt.tile([128, 2, QTOK], f32, tag="qt", name="qt")
                    qacc = [ppp.tile([128, QTOK], f32, tag="pp", name=f"qacc{_i}")
                            for _i in range(2)]
                    for c in range(8):
                        qs = pwslab.tile([128, GD], f32, tag="wslab", name="qs")
                        nc.sync.dma_start(
                            out=rc(qs),
                            in_=rc(wq_h[c * 128 : (c + 1) * 128,
                                        g * GD : (g + 1) * GD]),
                        )
                        for i in range(2):
                            nc.tensor.matmul(
                                qacc[i][:, :],
                                mm(qs[:, i * 128 : (i + 1) * 128]),
                                mm(srcq_sb[:, c, :]),
                                start=(c == 0),
                                stop=(c == 7),
                            )
                    for i in range(2):
                        nc.vector.tensor_scalar_add(
                            rc(qt[:, i, :]), qacc[i][:, :],
                            bq_sb[:, g * 2 + i : g * 2 + i + 1],
                        )
                    # -- K / V projections (streamed over token blocks) --
                    wkg = pwatt.tile([128, 8, GD], f32, tag="wkh", name="wkg")
                    nc.sync.dma_start(
                        out=rc(wkg),
                        in_=rc(wk_h[:, g * GD : (g + 1) * GD].rearrange(
                            "(c p) n -> p c n", p=128
                        )),
                    )
                    wvg = pwatt.tile([128, 8, GD], f32, tag="wvh", name="wvg")
                    nc.sync.dma_start(
                        out=rc(wvg),
                        in_=rc(wv_h[:, g * GD : (g + 1) * GD].rearrange(
                            "(c p) n -> p c n", p=128
                        )),
                    )
                    kt = pkt.tile([128, 2, S], f32, tag="kt", name="kt")
                    vp = pvp.tile([128, 16, VROW], f32, tag="vp", name="vp")
                    # ones columns at h*65 for h=0..3, plus trailing col 260
                    nc.vector.memset(
                        vp[:, :, 0 : 65 * GH].rearrange(
                            "p t (h u) -> p t h u", u=65
                        )[:, :, :, 0:1],
                        1.0,
                    )
                    nc.vector.memset(vp[:, :, VROW - 1 : VROW], 1.0)
                    for blk in range(NBLK):
                        xb = pxblk.tile([128, 8, TB], f32, tag="xblk", name="xb")
                        nc.sync.dma_start(
                            out=rc(xb),
                            in_=rc(src_dram[:, blk * TB : (blk + 1) * TB]
                                   .rearrange("(c p) t -> p c t", p=128)),
                        )
                        for dt in range(2):
                            kacc = ppp.tile([128, TB], f32, tag="pp",
                                            name="kacc")
                            for c in range(8):
                                nc.tensor.matmul(
                                    kacc[:, :],
                                    mm(wkg[:, c, dt * 128 : (dt + 1) * 128]),
                                    mm(xb[:, c, :]),
                                    start=(c == 0),
                                    stop=(c == 7),
                                )
                            nc.vector.tensor_scalar_add(
                                rc(kt[:, dt, blk * TB : (blk + 1) * TB]),
                                kacc[:, :],
                                bk_sb[:, g * 2 + dt : g * 2 + dt + 1],
                            )
                        for tt in range(TB // 128):
                            vacc = ppp.tile([128, GD], f32, tag="pp",
                                            name="vacc")
                            for c in range(8):
                                nc.tensor.matmul(
                                    vacc[:, :],
                                    mm(xb[:, c, tt * 128 : (tt + 1) * 128]),
                                    mm(wvg[:, c, :]),
                                    start=(c == 0),
                                    stop=(c == 7),
                                )
                            j = blk * (TB // 128) + tt
                            dst = vp[:, j, 1 : 1 + 65 * GH].rearrange(
                                "p (h u) -> p h u", u=65
                            )[:, :, 0:HD]
                            nc.vector.tensor_tensor(
                                out=rc(dst),
                                in0=vacc.rearrange("p (h u) -> p h u", u=HD),
                                in1=bv_sb[:, g * GD : (g + 1) * GD].rearrange(
                                    "p (h u) -> p h u", u=HD
                                ),
                                op=Alu.add,
                            )
                    # -- attention for the 4 heads of this group --
                    for h in range(GH):
                        ha = g * GH + h
                        par = h % 2
                        dt = h // 2
                        pc = ppc.tile([128, QTOK], f32, tag="pc", name="pc")
                        vlo = h * 65 + 1   # [v0..v63, one] -> denom row 64
                        for jg in range(8):
                            ps = pps.tile([128, 2, QTOK], f32, tag="ps",
                                          name="ps")
                            for js in range(2):
                                j = jg * 2 + js
                                nc.tensor.matmul(
                                    ps[:, js, :],
                                    mm(kt[par * 64 : par * 64 + 64, dt,
                                          j * 128 : (j + 1) * 128]),
                                    mm(qt[par * 64 : par * 64 + 64, dt, :]),
                                    start=True,
                                    stop=True,
                                )
                            et = pexp.tile([128, 2, QTOK], f32, tag="e",
                                           name="et")
                            nc.scalar.activation(
                                out=rc(et), in_=ps, func=AF.Exp,
                                scale=1.0 / (HD ** 0.5),
                            )
                            for js in range(2):
                                j = jg * 2 + js
                                nc.tensor.matmul(
                                    pc[0:65, :],
                                    mm(vp[:, j, vlo : vlo + 65]),
                                    mm(et[:, js, :]),
                                    start=(jg == 0 and js == 0),
                                    stop=(jg == 7 and js == 1),
                                )
                        # normalize by the denominator row (psum row 64)
                        rd = psq.tile([128, QTOK], f32, tag="sq", name="rd")
                        with nc.allow_low_precision("f32r bcast feed"):
                            nc.vector.reciprocal(
                                out=rc(rd[64:65, :]), in_=pc[64:65, :]
                            )
                        rb = pps.tile([128, 2, QTOK], f32, tag="ps", name="rb")
                        nc.tensor.matmul(
                            rb[0:64, 0, :],
                            mm(ones[64:65, 0:64]),
                            mm(rd[64:65, :]),
                            start=True,
                            stop=True,
                        )
                        cu = pexp.tile([128, QTOK], f32, tag="cu", name="cu")
                        nc.vector.tensor_copy(out=cu[0:64, :], in_=pc[0:64, :])
                        if par == 0:
                            nc.vector.tensor_tensor(
                                out=ctx_tile[0:64, ha // 2, :],
                                in0=cu[0:64, :],
                                in1=rb[0:64, 0, :],
                                op=Alu.mult,
                            )
                        else:
                            # engines can't cross partitions: normalize at
                            # base 0, then DMA-shift into partitions 64+
                            cn = psq.tile([128, QTOK], f32, tag="sq",
                                          name="cn")
                            nc.vector.tensor_tensor(
                                out=cn[0:64, :],
                                in0=cu[0:64, :],
                                in1=rb[0:64, 0, :],
                                op=Alu.mult,
                            )
                            nc.sync.dma_start(
                                out=ctx_tile[64:128, ha // 2, :],
                                in_=cn[0:64, :],
                            )

            def layernorm(x1, gname, out_tile, dma_out=None):
                """out = LN(x1) * g + b, normalizing over the partition dim."""
                g_sb, b_sb = sb[f"{gname}_g"], sb[f"{gname}_b"]
                sum_ps = ppp.tile([1, QTOK], f32, tag="pp")
                sq_ps = ppp.tile([1, QTOK], f32, tag="pp")
                for i in range(8):
                    nc.tensor.matmul(
                        sum_ps[:, :], mm(ones[:, 0:1]), mm(x1[:, i, :]),
                        start=(i == 0), stop=(i == 7),
                    )
                for i in range(8):
                    sqt = psq.tile([128, QTOK], f32, tag="sq")
                    nc.vector.tensor_mul(rc(sqt), x1[:, i, :], x1[:, i, :])
                    nc.tensor.matmul(
                        sq_ps[:, :], mm(ones[:, 0:1]), mm(sqt[:, :]),
                        start=(i == 0), stop=(i == 7),
                    )
                s_sb = pstat.tile([1, 2, QTOK], f32, tag="s_sb")
                nc.vector.tensor_copy(out=rc(s_sb[0:1, 0, :]), in_=sum_ps[:, :])
                nc.vector.tensor_copy(out=rc(s_sb[0:1, 1, :]), in_=sq_ps[:, :])
                bps = pps.tile([128, 2, QTOK], f32, tag="ps")
                for k in range(2):
                    nc.tensor.matmul(
                        bps[:, k, :], mm(ones[0:1, 0:128]), mm(s_sb[0:1, k, :]),
                        start=True, stop=True,
                    )
                meanb = pstat.tile([128, QTOK], f32, tag="meanb")
                nc.vector.tensor_scalar_mul(meanb, bps[:, 0, :], 1.0 / D)
                varb = pstat.tile([128, QTOK], f32, tag="varb")
                nc.vector.tensor_scalar_mul(varb, bps[:, 1, :], 1.0 / D)
                tmpb = pstat.tile([128, QTOK], f32, tag="tmpb")
                nc.vector.tensor_mul(tmpb, meanb, meanb)
                nc.vector.tensor_sub(varb, varb, tmpb)
                nc.vector.tensor_scalar_add(varb, varb, EPS)
                nc.vector.reciprocal(out=tmpb, in_=varb)
                rstdb = pstat.tile([128, QTOK], f32, tag="rstdb")
                nc.scalar.activation(out=rstdb, in_=tmpb, func=AF.Sqrt)
                for i in range(8):
                    t1 = psq.tile([128, QTOK], f32, tag="sq")
                    nc.vector.tensor_sub(t1, x1[:, i, :], meanb)
                    nc.vector.tensor_mul(t1, t1, rstdb)
                    nc.vector.tensor_scalar(
                        out=rc(out_tile[:, i, :]), in0=t1,
                        scalar1=g_sb[:, i : i + 1], scalar2=b_sb[:, i : i + 1],
                        op0=Alu.mult, op1=Alu.add,
                    )
                    if dma_out is not None:
                        nc.sync.dma_start(
                            out=dma_out[:, i, :], in_=out_tile[:, i, :]
                        )

            def eight_psums():
                """8 one-bank [128,512] accumulators spanning all three pools."""
                a = [ppp.tile([128, QTOK], f32, tag="pp", name=f"fa{_i}") for _i in range(2)]
                b = [ppc.tile([128, QTOK], f32, tag="pc", name=f"fb{_i}") for _i in range(2)]
                c_ = [pps.tile([128, 2, QTOK], f32, tag="ps", name=f"fc{_i}") for _i in range(2)]
                return [a[0][:, :], a[1][:, :], b[0][:, :], b[1][:, :],
                        c_[0][:, 0, :], c_[0][:, 1, :], c_[1][:, 0, :], c_[1][:, 1, :]]

            # ================= self-attention =================
            ctx1 = pctx.tile([128, 8, QTOK], f32, tag="ctx")
            attention("sa", xT, xq_sb, ctx1)
            x1 = pxa.tile([128, 8, QTOK], f32, tag="xa")
            nc.vector.tensor_add(rc(x1), xq_sb, ctx1)
            x2 = pxa.tile([128, 8, QTOK], f32, tag="xa")
            layernorm(x1, "ln1", x2)

            # ================= cross-attention =================
            ctx2 = pctx.tile([128, 8, QTOK], f32, tag="ctx")
            attention("ca", encT, x2, ctx2)
            x3p = pxa.tile([128, 8, QTOK], f32, tag="xa")
            nc.vector.tensor_add(rc(x3p), x2, ctx2)
            x3 = pqt.tile([128, 8, QTOK], f32, tag="qt")
            layernorm(x3p, "ln2", x3)

            # ================= feed-forward =================
            h2acc = pctx.tile([128, 8, QTOK], f32, tag="ctx")
            for qtr in range(4):
                h1q = pkt.tile([128, 8, QTOK], f32, tag="kt")
                h1accs = eight_psums()
                for c in range(8):
                    f1s = pwslab.tile([128, 1024], f32, tag="wslab")
                    nc.sync.dma_start(
                        out=rc(f1s),
                        in_=rc(fc1_w[c * 128 : (c + 1) * 128,
                                     qtr * 1024 : (qtr + 1) * 1024]),
                    )
                    for f in range(8):
                        nc.tensor.matmul(
                            h1accs[f],
                            mm(f1s[:, f * 128 : (f + 1) * 128]),
                            mm(x3[:, c, :]),
                            start=(c == 0),
                            stop=(c == 7),
                        )
                for f in range(8):
                    nc.scalar.activation(
                        out=rc(h1q[:, f, :]), in_=h1accs[f], func=AF.Relu,
                        bias=fc1b_sb[:, qtr * 8 + f : qtr * 8 + f + 1],
                    )
                h2accs = eight_psums()
                for f in range(8):
                    f2s = pwslab.tile([128, 1024], f32, tag="wslab")
                    nc.sync.dma_start(
                        out=rc(f2s),
                        in_=rc(fc2_w[qtr * 1024 + f * 128
                                     : qtr * 1024 + (f + 1) * 128, :]),
                    )
                    for i in range(8):
                        nc.tensor.matmul(
                            h2accs[i],
                            mm(f2s[:, i * 128 : (i + 1) * 128]),
                            mm(h1q[:, f, :]),
                            start=(f == 0),
                            stop=(f == 7),
                        )
                for i in range(8):
                    if qtr == 0:
                        nc.vector.tensor_copy(out=h2acc[:, i, :], in_=h2accs[i])
                    else:
                        nc.vector.tensor_tensor(
                            out=h2acc[:, i, :], in0=h2acc[:, i, :],
                            in1=h2accs[i], op=Alu.add,
                        )
            x4p = pxa.tile([128, 8, QTOK], f32, tag="xa")
            for i in range(8):
                nc.vector.tensor_scalar_add(
                    h2acc[:, i, :], h2acc[:, i, :], fc2b_sb[:, i : i + 1]
                )
            nc.vector.tensor_add(rc(x4p), x3, h2acc)
            x4 = pxa.tile([128, 8, QTOK], f32, tag="xa")
            layernorm(x4p, "ln3", x4,
                      dma_out=outT.rearrange("(i p) t -> p i t", p=128))

    nc.compile()
    return nc


def _get_program(mode=DTYPE_MODE):
    if mode not in _PROGRAM_CACHE:
        _PROGRAM_CACHE[mode] = _build_program(mode)
    return _PROGRAM_CACHE[mode]


def _make_in_maps(inputs):
    f = np.float32

    def colmajor8(v):  # [1024] -> [128, 8] with [p, i] = v[i*128+p]
        return np.ascontiguousarray(v.reshape(8, 128).T.astype(f))

    shared = {}
    for p in ("sa", "ca"):
        for wn in ("wq", "wk", "wv"):
            shared[f"{p}_{wn}"] = np.ascontiguousarray(inputs[f"{p}_{wn}"], dtype=f)
        for bn in ("bq", "bk"):
            shared[f"{p}_{bn}"] = colmajor8(np.asarray(inputs[f"{p}_{bn}"]))
        shared[f"{p}_bv"] = np.ascontiguousarray(
            np.broadcast_to(np.asarray(inputs[f"{p}_bv"], dtype=f), (128, D))
        )
    shared["fc1_w"] = np.ascontiguousarray(inputs["fc1_w"], dtype=f)
    shared["fc2_w"] = np.ascontiguousarray(inputs["fc2_w"], dtype=f)
    shared["fc1_b"] = np.ascontiguousarray(
        np.asarray(inputs["fc1_b"]).reshape(32, 128).T.astype(f)
    )
    shared["fc2_b"] = colmajor8(np.asarray(inputs["fc2_b"]))
    for i in (1, 2, 3):
        shared[f"ln{i}_g"] = colmajor8(np.asarray(inputs[f"ln{i}_g"]))
        shared[f"ln{i}_b"] = colmajor8(np.asarray(inputs[f"ln{i}_b"]))

    hs = np.asarray(inputs["hidden_states"], dtype=f)
    enc = np.asarray(inputs["encoder_hidden_states"], dtype=f)
    in_maps = []
    for c in range(NCORES):
        b, q0 = c // 4, (c % 4) * QTOK
        m = dict(shared)
        m["xT"] = np.ascontiguousarray(hs[b].T)
        m["xqT"] = np.ascontiguousarray(hs[b, q0 : q0 + QTOK, :].T)
        m["encT"] = np.ascontiguousarray(enc[b].T)
        in_maps.append(m)
    return in_maps


def kernel(**inputs):
    from concourse.bass_utils import run_bass_kernel_spmd

    nc = _get_program()
    in_maps = _make_in_maps(inputs)
    res = run_bass_kernel_spmd(nc, in_maps, core_ids=list(range(NCORES)))
    out = np.empty((2, S, D), np.float32)
    for c in range(NCORES):
        b, q0 = c // 4, (c % 4) * QTOK
        out[b, q0 : q0 + QTOK, :] = res.results[c]["outT"].T
    return out


# revision 24
# speedup vs baseline: 1.3103x; 1.1945x over previous
"""Trainium2 Bass kernel for nn_DecoderLayer (self-attn + cross-attn + FFN).

Sharding: 8 cores = 2 batches x 4 query-blocks of 512 tokens (data/sequence
parallel, zero collectives). Each core recomputes the full K/V for its batch
and runs its 512 queries through the whole layer.

Layouts (per core):
  activations transposed [D, tok]; scores computed transposed [k, q] so the
  softmax denominator comes free via a ones-column appended to V; LayerNorm
  over the partition dim via ones-vector matmuls + PE broadcast.
Matmuls run in float32r (1 cyc/row vs 4 for fp32) unless DTYPE_MODE='f32'.
"""

import sys

if "/opt/trn_rl_repo" not in sys.path:
    sys.path.insert(0, "/opt/trn_rl_repo")

import numpy as np

D = 1024
S = 2048
QTOK = 512          # queries per core
H = 16
HD = 64
FFN = 4096
EPS = 1e-5
NCORES = 8
NG = 4              # head groups per attention
GH = 4              # heads per group
GD = GH * HD        # 256 dout per group
NBLK = 8            # token blocks for K/V projection streaming
TB = S // NBLK      # 256
VROW = GH * (HD + 1) + 1   # 261 cols per k-tile in V'' (ones interleaved)

DTYPE_MODE = "f32r"  # 'f32r' | 'f32'

_PROGRAM_CACHE = {}


def _build_program(mode=DTYPE_MODE):
    import contextlib

    import concourse.bacc as bacc
    import concourse.bass as bass_mod
    import concourse.tile as tile
    from concourse import mybir

    f32 = mybir.dt.float32
    f32r = mybir.dt.float32r
    AF = mybir.ActivationFunctionType
    Alu = mybir.AluOpType

    def mm(ap):
        """cast a matmul operand to the fast dtype"""
        return ap.bitcast(f32r) if mode == "f32r" else ap

    rc = mm  # producers of matmul-consumed data must emit f32r-rounded output

    nc = bacc.Bacc("TRN2", target_bir_lowering=False)

    # ---- DRAM parameters (per-core data supplied via in_maps) ----
    def din(name, shape):
        return nc.declare_dram_parameter(name, list(shape), f32, isOutput=False)

    xT = din("xT", (D, S))            # hidden[b].T
    xqT = din("xqT", (D, QTOK))       # hidden[b, q0:q0+512].T
    encT = din("encT", (D, S))        # encoder[b].T
    w = {}
    for p in ("sa", "ca"):
        for wn in ("wq", "wk", "wv"):
            w[f"{p}_{wn}"] = din(f"{p}_{wn}", (D, D))
        for bn in ("bq", "bk"):
            w[f"{p}_{bn}"] = din(f"{p}_{bn}", (128, 8))   # reshaped (8,128).T
        w[f"{p}_bv"] = din(f"{p}_bv", (128, D))           # row-broadcast
    fc1_w = din("fc1_w", (D, FFN))
    fc2_w = din("fc2_w", (FFN, D))
    fc1_b = din("fc1_b", (128, 32))
    fc2_b = din("fc2_b", (128, 8))
    for i in (1, 2, 3):
        w[f"ln{i}_g"] = din(f"ln{i}_g", (128, 8))
        w[f"ln{i}_b"] = din(f"ln{i}_b", (128, 8))
    outT = nc.declare_dram_parameter("outT", [D, QTOK], f32, isOutput=True)

    with tile.TileContext(nc) as tc:
        with contextlib.ExitStack() as ctx:
            consts = ctx.enter_context(tc.tile_pool(name="consts", bufs=1))
            pkt = ctx.enter_context(tc.tile_pool(name="pkt", bufs=1))
            pvp = ctx.enter_context(tc.tile_pool(name="pvp", bufs=1))
            pqt = ctx.enter_context(tc.tile_pool(name="pqt", bufs=1))
            pxa = ctx.enter_context(tc.tile_pool(name="pxa", bufs=2))
            pxblk = ctx.enter_context(tc.tile_pool(name="pxblk", bufs=2))
            pwatt = ctx.enter_context(tc.tile_pool(name="pwatt", bufs=1))
            pwslab = ctx.enter_context(tc.tile_pool(name="pwslab", bufs=3))
            pctx = ctx.enter_context(tc.tile_pool(name="pctx", bufs=1))
            pexp = ctx.enter_context(tc.tile_pool(name="pexp", bufs=3))
            psq = ctx.enter_context(tc.tile_pool(name="psq", bufs=2))
            pstat = ctx.enter_context(tc.tile_pool(name="pstat", bufs=1))
            ppp = ctx.enter_context(tc.tile_pool(name="ppp", bufs=2, space="PSUM"))
            pps = ctx.enter_context(tc.tile_pool(name="pps", bufs=2, space="PSUM"))
            ppc = ctx.enter_context(tc.tile_pool(name="ppc", bufs=2, space="PSUM"))

            # ---- constants ----
            ones = consts.tile([128, 128], f32, tag="ones")
            nc.vector.memset(ones, 1.0)
            xq_sb = consts.tile([128, 8, QTOK], f32, tag="xq")
            xqT_v = xqT.rearrange("(c p) t -> p c t", p=128)
            for _c in range(8):
                nc.sync.dma_start(
                    out=rc(xq_sb[:, _c, :]), in_=rc(xqT_v[:, _c, :])
                )
            sb = {}
            for name, hnd in w.items():
                if name.endswith(("wq", "wk", "wv")):
                    continue
                sb[name] = consts.tile([128, hnd.shape[1]], f32, tag=name, name=name)
                nc.sync.dma_start(out=sb[name], in_=hnd[:, :])
            fc1b_sb = consts.tile([128, 32], f32, tag="fc1b")
            nc.sync.dma_start(out=fc1b_sb, in_=fc1_b[:, :])
            fc2b_sb = consts.tile([128, 8], f32, tag="fc2b")
            nc.sync.dma_start(out=fc2b_sb, in_=fc2_b[:, :])

            def attention(pfx, src_dram, srcq_sb, ctx_tile):
                """One MHA: Q from srcq_sb [128,8,512], K/V from src_dram [D,S].
                Writes normalized ctx^T into ctx_tile [128,8,512]."""
                wq_h, wk_h, wv_h = w[f"{pfx}_wq"], w[f"{pfx}_wk"], w[f"{pfx}_wv"]
                bq_sb, bk_sb, bv_sb = sb[f"{pfx}_bq"], sb[f"{pfx}_bk"], sb[f"{pfx}_bv"]
                for g in range(NG):
                    # -- Q projection for this group (2 dtiles of 128) --
                    qt = pqt.tile([128, 2, QTOK], f32, tag="qt", name="qt")
                    qacc = [ppp.tile([128, QTOK], f32, tag="pp", name=f"qacc{_i}")
                            for _i in range(2)]
                    for c in range(8):
                        qs = pwslab.tile([128, GD], f32, tag="wslab", name="qs")
                        nc.sync.dma_start(
                            out=rc(qs),
                            in_=rc(wq_h[c * 128 : (c + 1) * 128,
                                        g * GD : (g + 1) * GD]),
                        )
                        for i in range(2):
                            nc.tensor.matmul(
                                qacc[i][:, :],
                                mm(qs[:, i * 128 : (i + 1) * 128]),
                                mm(srcq_sb[:, c, :]),
                                start=(c == 0),
                                stop=(c == 7),
                            )
                    for i in range(2):
                        nc.vector.tensor_scalar_add(
                            rc(qt[:, i, :]), qacc[i][:, :],
                            bq_sb[:, g * 2 + i : g * 2 + i + 1],
                        )
                    # -- K / V projections (streamed over token blocks) --
                    wkg = pwatt.tile([128, 8, GD], f32, tag="wkh", name="wkg")
                    nc.sync.dma_start(
                        out=rc(wkg),
                        in_=rc(wk_h[:, g * GD : (g + 1) * GD].rearrange(
                            "(c p) n -> p c n", p=128
                        )),
                    )
                    wvg = pwatt.tile([128, 8, GD], f32, tag="wvh", name="wvg")
                    nc.sync.dma_start(
                        out=rc(wvg),
                        in_=rc(wv_h[:, g * GD : (g + 1) * GD].rearrange(
                            "(c p) n -> p c n", p=128
                        )),
                    )
                    kt = pkt.tile([128, 2, S], f32, tag="kt", name="kt")
                    vp = pvp.tile([128, 16, VROW], f32, tag="vp", name="vp")
                    # ones columns at h*65 for h=0..3, plus trailing col 260
                    nc.vector.memset(
                        vp[:, :, 0 : 65 * GH].rearrange(
                            "p t (h u) -> p t h u", u=65
                        )[:, :, :, 0:1],
                        1.0,
                    )
                    nc.vector.memset(vp[:, :, VROW - 1 : VROW], 1.0)
                    for blk in range(NBLK):
                        xb = pxblk.tile([128, 8, TB], f32, tag="xblk", name="xb")
                        nc.sync.dma_start(
                            out=rc(xb),
                            in_=rc(src_dram[:, blk * TB : (blk + 1) * TB]
                                   .rearrange("(c p) t -> p c t", p=128)),
                        )
                        for dt in range(2):
                            kacc = ppp.tile([128, TB], f32, tag="pp",
                                            name="kacc")
                            for c in range(8):
                                nc.tensor.matmul(
                                    kacc[:, :],
                                    mm(wkg[:, c, dt * 128 : (dt + 1) * 128]),
                                    mm(xb[:, c, :]),
                                    start=(c == 0),
                                    stop=(c == 7),
                                )
                            nc.vector.tensor_scalar_add(
                                rc(kt[:, dt, blk * TB : (blk + 1) * TB]),
                                kacc[:, :],
                                bk_sb[:, g * 2 + dt : g * 2 + dt + 1],
                            )
                        for tt in range(TB // 128):
                            vacc = ppp.tile([128, GD], f32, tag="pp",
                                            name="vacc")
                            for c in range(8):
                                nc.tensor.matmul(
                                    vacc[:, :],
                                    mm(xb[:, c, tt * 128 : (tt + 1) * 128]),
                                    mm(wvg[:, c, :]),
                                    start=(c == 0),
                                    stop=(c == 7),
                                )
                            j = blk * (TB // 128) + tt
                            dst = vp[:, j, 1 : 1 + 65 * GH].rearrange(
                                "p (h u) -> p h u", u=65
                            )[:, :, 0:HD]
                            nc.vector.tensor_tensor(
                                out=rc(dst),
                                in0=vacc.rearrange("p (h u) -> p h u", u=HD),
                                in1=bv_sb[:, g * GD : (g + 1) * GD].rearrange(
                                    "p (h u) -> p h u", u=HD
                                ),
                                op=Alu.add,
                            )
                    # -- attention for the 4 heads of this group --
                    for h in range(GH):
                        ha = g * GH + h
                        par = h % 2
                        dt = h // 2
                        pc = ppc.tile([128, QTOK], f32, tag="pc", name="pc")
                        vlo = h * 65 + 1   # [v0..v63, one] -> denom row 64
                        for jg in range(8):
                            ps = pps.tile([128, 2, QTOK], f32, tag="ps",
                                          name="ps")
                            for js in range(2):
                                j = jg * 2 + js
                                nc.tensor.matmul(
                                    ps[:, js, :],
                                    mm(kt[par * 64 : par * 64 + 64, dt,
                                          j * 128 : (j + 1) * 128]),
                                    mm(qt[par * 64 : par * 64 + 64, dt, :]),
                                    start=True,
                                    stop=True,
                                )
                            et = pexp.tile([128, 2, QTOK], f32, tag="e",
                                           name="et")
                            nc.scalar.activation(
                                out=rc(et), in_=ps, func=AF.Exp,
                                scale=1.0 / (HD ** 0.5),
                            )
                            for js in range(2):
                                j = jg * 2 + js
                                nc.tensor.matmul(
                                    pc[0:65, :],
                                    mm(vp[:, j, vlo : vlo + 65]),
                                    mm(et[:, js, :]),
                                    start=(jg == 0 and js == 0),
                                    stop=(jg == 7 and js == 1),
                                )
                        # normalize by the denominator row (psum row 64).
                        # the reciprocal broadcast runs as a partition-step-0
                        # DMA so it stays OFF the in-order PE stream: the PE
                        # flows from the last ctx matmul straight into the
                        # next head/group instead of stalling on a bcast mm.
                        rd = psq.tile([128, QTOK], f32, tag="sq", name="rd")
                        nc.vector.reciprocal(
                            out=rd[64:65, :], in_=pc[64:65, :]
                        )
                        # hw partition_broadcast reads physical partition 0:
                        # DMA-shift the value down first (off the PE stream)
                        nc.sync.dma_start(out=rd[0:1, :], in_=rd[64:65, :])
                        rbs = pexp.tile([128, QTOK], f32, tag="rbs",
                                        name="rbs")
                        nc.gpsimd.partition_broadcast(
                            out_ap=rbs[0:64, :], in_ap=rd[0:64, :],
                            channels=64,
                        )
                        cu = pexp.tile([128, QTOK], f32, tag="cu", name="cu")
                        nc.vector.tensor_copy(out=cu[0:64, :], in_=pc[0:64, :])
                        if par == 0:
                            nc.vector.tensor_tensor(
                                out=ctx_tile[0:64, ha // 2, :],
                                in0=cu[0:64, :],
                                in1=rbs[0:64, :],
                                op=Alu.mult,
                            )
                        else:
                            # engines can't cross partitions: normalize at
                            # base 0, then DMA-shift into partitions 64+
                            cn = psq.tile([128, QTOK], f32, tag="sq",
                                          name="cn")
                            nc.vector.tensor_tensor(
                                out=cn[0:64, :],
                                in0=cu[0:64, :],
                                in1=rbs[0:64, :],
                                op=Alu.mult,
                            )
                            nc.sync.dma_start(
                                out=ctx_tile[64:128, ha // 2, :],
                                in_=cn[0:64, :],
                            )

            def layernorm(x1, gname, out_tile, dma_out=None):
                """out = LN(x1) * g + b, normalizing over the partition dim."""
                g_sb, b_sb = sb[f"{gname}_g"], sb[f"{gname}_b"]
                sum_ps = ppp.tile([1, QTOK], f32, tag="pp")
                sq_ps = ppp.tile([1, QTOK], f32, tag="pp")
                for i in range(8):
                    nc.tensor.matmul(
                        sum_ps[:, :], mm(ones[:, 0:1]), mm(x1[:, i, :]),
                        start=(i == 0), stop=(i == 7),
                    )
                for i in range(8):
                    sqt = psq.tile([128, QTOK], f32, tag="sq")
                    nc.vector.tensor_mul(rc(sqt), x1[:, i, :], x1[:, i, :])
                    nc.tensor.matmul(
                        sq_ps[:, :], mm(ones[:, 0:1]), mm(sqt[:, :]),
                        start=(i == 0), stop=(i == 7),
                    )
                s_sb = pstat.tile([1, 2, QTOK], f32, tag="s_sb")
                nc.vector.tensor_copy(out=rc(s_sb[0:1, 0, :]), in_=sum_ps[:, :])
                nc.vector.tensor_copy(out=rc(s_sb[0:1, 1, :]), in_=sq_ps[:, :])
                bps = pps.tile([128, 2, QTOK], f32, tag="ps")
                for k in range(2):
                    nc.tensor.matmul(
                        bps[:, k, :], mm(ones[0:1, 0:128]), mm(s_sb[0:1, k, :]),
                        start=True, stop=True,
                    )
                meanb = pstat.tile([128, QTOK], f32, tag="meanb")
                nc.vector.tensor_scalar_mul(meanb, bps[:, 0, :], 1.0 / D)
                varb = pstat.tile([128, QTOK], f32, tag="varb")
                nc.vector.tensor_scalar_mul(varb, bps[:, 1, :], 1.0 / D)
                tmpb = pstat.tile([128, QTOK], f32, tag="tmpb")
                nc.vector.tensor_mul(tmpb, meanb, meanb)
                nc.vector.tensor_sub(varb, varb, tmpb)
                nc.vector.tensor_scalar_add(varb, varb, EPS)
                nc.vector.reciprocal(out=tmpb, in_=varb)
                rstdb = pstat.tile([128, QTOK], f32, tag="rstdb")
                nc.scalar.activation(out=rstdb, in_=tmpb, func=AF.Sqrt)
                for i in range(8):
                    t1 = psq.tile([128, QTOK], f32, tag="sq")
                    nc.vector.tensor_sub(t1, x1[:, i, :], meanb)
                    nc.vector.tensor_mul(t1, t1, rstdb)
                    nc.vector.tensor_scalar(
                        out=rc(out_tile[:, i, :]), in0=t1,
                        scalar1=g_sb[:, i : i + 1], scalar2=b_sb[:, i : i + 1],
                        op0=Alu.mult, op1=Alu.add,
                    )
                    if dma_out is not None:
                        nc.sync.dma_start(
                            out=dma_out[:, i, :], in_=out_tile[:, i, :]
                        )

            def eight_psums():
                """8 one-bank [128,512] accumulators spanning all three pools."""
                a = [ppp.tile([128, QTOK], f32, tag="pp", name=f"fa{_i}") for _i in range(2)]
                b = [ppc.tile([128, QTOK], f32, tag="pc", name=f"fb{_i}") for _i in range(2)]
                c_ = [pps.tile([128, 2, QTOK], f32, tag="ps", name=f"fc{_i}") for _i in range(2)]
                return [a[0][:, :], a[1][:, :], b[0][:, :], b[1][:, :],
                        c_[0][:, 0, :], c_[0][:, 1, :], c_[1][:, 0, :], c_[1][:, 1, :]]

            # ================= self-attention =================
            ctx1 = pctx.tile([128, 8, QTOK], f32, tag="ctx")
            attention("sa", xT, xq_sb, ctx1)
            x1 = pxa.tile([128, 8, QTOK], f32, tag="xa")
            nc.vector.tensor_add(rc(x1), xq_sb, ctx1)
            x2 = pxa.tile([128, 8, QTOK], f32, tag="xa")
            layernorm(x1, "ln1", x2)

            # ================= cross-attention =================
            ctx2 = pctx.tile([128, 8, QTOK], f32, tag="ctx")
            attention("ca", encT, x2, ctx2)
            x3p = pxa.tile([128, 8, QTOK], f32, tag="xa")
            nc.vector.tensor_add(rc(x3p), x2, ctx2)
            x3 = pqt.tile([128, 8, QTOK], f32, tag="qt")
            layernorm(x3p, "ln2", x3)

            # ================= feed-forward =================
            h2acc = pctx.tile([128, 8, QTOK], f32, tag="ctx")
            for qtr in range(4):
                h1q = pkt.tile([128, 8, QTOK], f32, tag="kt")
                h1accs = eight_psums()
                for c in range(8):
                    f1s = pwslab.tile([128, 1024], f32, tag="wslab")
                    nc.sync.dma_start(
                        out=rc(f1s),
                        in_=rc(fc1_w[c * 128 : (c + 1) * 128,
                                     qtr * 1024 : (qtr + 1) * 1024]),
                    )
                    for f in range(8):
                        nc.tensor.matmul(
                            h1accs[f],
                            mm(f1s[:, f * 128 : (f + 1) * 128]),
                            mm(x3[:, c, :]),
                            start=(c == 0),
                            stop=(c == 7),
                        )
                for f in range(8):
                    nc.scalar.activation(
                        out=rc(h1q[:, f, :]), in_=h1accs[f], func=AF.Relu,
                        bias=fc1b_sb[:, qtr * 8 + f : qtr * 8 + f + 1],
                    )
                h2accs = eight_psums()
                for f in range(8):
                    f2s = pwslab.tile([128, 1024], f32, tag="wslab")
                    nc.sync.dma_start(
                        out=rc(f2s),
                        in_=rc(fc2_w[qtr * 1024 + f * 128
                                     : qtr * 1024 + (f + 1) * 128, :]),
                    )
                    for i in range(8):
                        nc.tensor.matmul(
                            h2accs[i],
                            mm(f2s[:, i * 128 : (i + 1) * 128]),
                            mm(h1q[:, f, :]),
                            start=(f == 0),
                            stop=(f == 7),
                        )
                for i in range(8):
                    if qtr == 0:
                        nc.vector.tensor_copy(out=h2acc[:, i, :], in_=h2accs[i])
                    else:
                        nc.vector.tensor_tensor(
                            out=h2acc[:, i, :], in0=h2acc[:, i, :],
                            in1=h2accs[i], op=Alu.add,
                        )
            x4p = pxa.tile([128, 8, QTOK], f32, tag="xa")
            for i in range(8):
                nc.vector.tensor_scalar_add(
                    h2acc[:, i, :], h2acc[:, i, :], fc2b_sb[:, i : i + 1]
                )
            nc.vector.tensor_add(rc(x4p), x3, h2acc)
            x4 = pxa.tile([128, 8, QTOK], f32, tag="xa")
            layernorm(x4p, "ln3", x4,
                      dma_out=outT.rearrange("(i p) t -> p i t", p=128))

    nc.compile()
    return nc


def _get_program(mode=DTYPE_MODE):
    if mode not in _PROGRAM_CACHE:
        _PROGRAM_CACHE[mode] = _build_program(mode)
    return _PROGRAM_CACHE[mode]


def _make_in_maps(inputs):
    f = np.float32

    def colmajor8(v):  # [1024] -> [128, 8] with [p, i] = v[i*128+p]
        return np.ascontiguousarray(v.reshape(8, 128).T.astype(f))

    shared = {}
    for p in ("sa", "ca"):
        for wn in ("wq", "wk", "wv"):
            shared[f"{p}_{wn}"] = np.ascontiguousarray(inputs[f"{p}_{wn}"], dtype=f)
        for bn in ("bq", "bk"):
            shared[f"{p}_{bn}"] = colmajor8(np.asarray(inputs[f"{p}_{bn}"]))
        shared[f"{p}_bv"] = np.ascontiguousarray(
            np.broadcast_to(np.asarray(inputs[f"{p}_bv"], dtype=f), (128, D))
        )
    shared["fc1_w"] = np.ascontiguousarray(inputs["fc1_w"], dtype=f)
    shared["fc2_w"] = np.ascontiguousarray(inputs["fc2_w"], dtype=f)
    shared["fc1_b"] = np.ascontiguousarray(
        np.asarray(inputs["fc1_b"]).reshape(32, 128).T.astype(f)
    )
    shared["fc2_b"] = colmajor8(np.asarray(inputs["fc2_b"]))
    for i in (1, 2, 3):
        shared[f"ln{i}_g"] = colmajor8(np.asarray(inputs[f"ln{i}_g"]))
        shared[f"ln{i}_b"] = colmajor8(np.asarray(inputs[f"ln{i}_b"]))

    hs = np.asarray(inputs["hidden_states"], dtype=f)
    enc = np.asarray(inputs["encoder_hidden_states"], dtype=f)
    in_maps = []
    for c in range(NCORES):
        b, q0 = c // 4, (c % 4) * QTOK
        m = dict(shared)
        m["xT"] = np.ascontiguousarray(hs[b].T)
        m["xqT"] = np.ascontiguousarray(hs[b, q0 : q0 + QTOK, :].T)
        m["encT"] = np.ascontiguousarray(enc[b].T)
        in_maps.append(m)
    return in_maps


def kernel(**inputs):
    from concourse.bass_utils import run_bass_kernel_spmd

    nc = _get_program()
    in_maps = _make_in_maps(inputs)
    res = run_bass_kernel_spmd(nc, in_maps, core_ids=list(range(NCORES)))
    out = np.empty((2, S, D), np.float32)
    for c in range(NCORES):
        b, q0 = c // 4, (c % 4) * QTOK
        out[b, q0 : q0 + QTOK, :] = res.results[c]["outT"].T
    return out


# revision 25
# speedup vs baseline: 1.3120x; 1.0012x over previous
"""Trainium2 Bass kernel for nn_DecoderLayer (self-attn + cross-attn + FFN).

Sharding: 8 cores = 2 batches x 4 query-blocks of 512 tokens (data/sequence
parallel, zero collectives). Each core recomputes the full K/V for its batch
and runs its 512 queries through the whole layer.

Layouts (per core):
  activations transposed [D, tok]; scores computed transposed [k, q] so the
  softmax denominator comes free via a ones-column appended to V; LayerNorm
  over the partition dim via ones-vector matmuls + PE broadcast.
Matmuls run in float32r (1 cyc/row vs 4 for fp32) unless DTYPE_MODE='f32'.
"""

import sys

if "/opt/trn_rl_repo" not in sys.path:
    sys.path.insert(0, "/opt/trn_rl_repo")

import numpy as np

D = 1024
S = 2048
QTOK = 512          # queries per core
H = 16
HD = 64
FFN = 4096
EPS = 1e-5
NCORES = 8
NG = 4              # head groups per attention
GH = 4              # heads per group
GD = GH * HD        # 256 dout per group
NBLK = 8            # token blocks for K/V projection streaming
TB = S // NBLK      # 256
VROW = GH * (HD + 1) + 1   # 261 cols per k-tile in V'' (ones interleaved)

DTYPE_MODE = "f32r"  # 'f32r' | 'f32'

_PROGRAM_CACHE = {}


def _build_program(mode=DTYPE_MODE):
    import contextlib

    import concourse.bacc as bacc
    import concourse.bass as bass_mod
    import concourse.tile as tile
    from concourse import mybir

    f32 = mybir.dt.float32
    f32r = mybir.dt.float32r
    AF = mybir.ActivationFunctionType
    Alu = mybir.AluOpType

    def mm(ap):
        """cast a matmul operand to the fast dtype"""
        return ap.bitcast(f32r) if mode == "f32r" else ap

    rc = mm  # producers of matmul-consumed data must emit f32r-rounded output

    nc = bacc.Bacc("TRN2", target_bir_lowering=False)

    # ---- DRAM parameters (per-core data supplied via in_maps) ----
    def din(name, shape):
        return nc.declare_dram_parameter(name, list(shape), f32, isOutput=False)

    xT = din("xT", (D, S))            # hidden[b].T
    xqT = din("xqT", (D, QTOK))       # hidden[b, q0:q0+512].T
    encT = din("encT", (D, S))        # encoder[b].T
    w = {}
    for p in ("sa", "ca"):
        for wn in ("wq", "wk", "wv"):
            w[f"{p}_{wn}"] = din(f"{p}_{wn}", (D, D))
        for bn in ("bq", "bk"):
            w[f"{p}_{bn}"] = din(f"{p}_{bn}", (128, 8))   # reshaped (8,128).T
        w[f"{p}_bv"] = din(f"{p}_bv", (128, D))           # row-broadcast
    fc1_w = din("fc1_w", (D, FFN))
    fc2_w = din("fc2_w", (FFN, D))
    fc1_b = din("fc1_b", (128, 32))
    fc2_b = din("fc2_b", (128, 8))
    for i in (1, 2, 3):
        w[f"ln{i}_g"] = din(f"ln{i}_g", (128, 8))
        w[f"ln{i}_b"] = din(f"ln{i}_b", (128, 8))
    outT = nc.declare_dram_parameter("outT", [D, QTOK], f32, isOutput=True)

    with tile.TileContext(nc) as tc:
        with contextlib.ExitStack() as ctx:
            consts = ctx.enter_context(tc.tile_pool(name="consts", bufs=1))
            pkt = ctx.enter_context(tc.tile_pool(name="pkt", bufs=1))
            pvp = ctx.enter_context(tc.tile_pool(name="pvp", bufs=1))
            pqt = ctx.enter_context(tc.tile_pool(name="pqt", bufs=1))
            pxa = ctx.enter_context(tc.tile_pool(name="pxa", bufs=2))
            pxblk = ctx.enter_context(tc.tile_pool(name="pxblk", bufs=2))
            pwatt = ctx.enter_context(tc.tile_pool(name="pwatt", bufs=1))
            pwslab = ctx.enter_context(tc.tile_pool(name="pwslab", bufs=3))
            pctx = ctx.enter_context(tc.tile_pool(name="pctx", bufs=1))
            pexp = ctx.enter_context(tc.tile_pool(name="pexp", bufs=3))
            psq = ctx.enter_context(tc.tile_pool(name="psq", bufs=2))
            pstat = ctx.enter_context(tc.tile_pool(name="pstat", bufs=1))
            ppp = ctx.enter_context(tc.tile_pool(name="ppp", bufs=2, space="PSUM"))
            pps = ctx.enter_context(tc.tile_pool(name="pps", bufs=2, space="PSUM"))
            ppc = ctx.enter_context(tc.tile_pool(name="ppc", bufs=2, space="PSUM"))

            # ---- constants ----
            ones = consts.tile([128, 128], f32, tag="ones")
            nc.vector.memset(ones, 1.0)
            xq_sb = consts.tile([128, 8, QTOK], f32, tag="xq")
            xqT_v = xqT.rearrange("(c p) t -> p c t", p=128)
            for _c in range(8):
                nc.sync.dma_start(
                    out=rc(xq_sb[:, _c, :]), in_=rc(xqT_v[:, _c, :])
                )
            sb = {}
            for name, hnd in w.items():
                if name.endswith(("wq", "wk", "wv")):
                    continue
                sb[name] = consts.tile([128, hnd.shape[1]], f32, tag=name, name=name)
                nc.sync.dma_start(out=sb[name], in_=hnd[:, :])
            fc1b_sb = consts.tile([128, 32], f32, tag="fc1b")
            nc.sync.dma_start(out=fc1b_sb, in_=fc1_b[:, :])
            fc2b_sb = consts.tile([128, 8], f32, tag="fc2b")
            nc.sync.dma_start(out=fc2b_sb, in_=fc2_b[:, :])

            def attention(pfx, src_dram, srcq_sb, ctx_tile):
                """One MHA: Q from srcq_sb [128,8,512], K/V from src_dram [D,S].
                Writes normalized ctx^T into ctx_tile [128,8,512]."""
                wq_h, wk_h, wv_h = w[f"{pfx}_wq"], w[f"{pfx}_wk"], w[f"{pfx}_wv"]
                bq_sb, bk_sb, bv_sb = sb[f"{pfx}_bq"], sb[f"{pfx}_bk"], sb[f"{pfx}_bv"]
                for g in range(NG):
                    # -- Q projection for this group (2 dtiles of 128) --
                    qt = pqt.tile([128, 2, QTOK], f32, tag="qt", name="qt")
                    qacc = [ppp.tile([128, QTOK], f32, tag="pp", name=f"qacc{_i}")
                            for _i in range(2)]
                    for c in range(8):
                        qs = pwslab.tile([128, GD], f32, tag="wslab", name="qs")
                        nc.sync.dma_start(
                            out=rc(qs),
                            in_=rc(wq_h[c * 128 : (c + 1) * 128,
                                        g * GD : (g + 1) * GD]),
                        )
                        for i in range(2):
                            nc.tensor.matmul(
                                qacc[i][:, :],
                                mm(qs[:, i * 128 : (i + 1) * 128]),
                                mm(srcq_sb[:, c, :]),
                                start=(c == 0),
                                stop=(c == 7),
                            )
                    for i in range(2):
                        nc.vector.tensor_scalar_add(
                            rc(qt[:, i, :]), qacc[i][:, :],
                            bq_sb[:, g * 2 + i : g * 2 + i + 1],
                        )
                    # -- K / V projections (streamed over token blocks) --
                    wkg = pwatt.tile([128, 8, GD], f32, tag="wkh", name="wkg")
                    nc.sync.dma_start(
                        out=rc(wkg),
                        in_=rc(wk_h[:, g * GD : (g + 1) * GD].rearrange(
                            "(c p) n -> p c n", p=128
                        )),
                    )
                    wvg = pwatt.tile([128, 8, GD], f32, tag="wvh", name="wvg")
                    nc.sync.dma_start(
                        out=rc(wvg),
                        in_=rc(wv_h[:, g * GD : (g + 1) * GD].rearrange(
                            "(c p) n -> p c n", p=128
                        )),
                    )
                    kt = pkt.tile([128, 2, S], f32, tag="kt", name="kt")
                    vp = pvp.tile([128, 16, VROW], f32, tag="vp", name="vp")
                    # ones columns at h*65 for h=0..3, plus trailing col 260
                    nc.vector.memset(
                        vp[:, :, 0 : 65 * GH].rearrange(
                            "p t (h u) -> p t h u", u=65
                        )[:, :, :, 0:1],
                        1.0,
                    )
                    nc.vector.memset(vp[:, :, VROW - 1 : VROW], 1.0)
                    for blk in range(NBLK):
                        xb = pxblk.tile([128, 8, TB], f32, tag="xblk", name="xb")
                        nc.sync.dma_start(
                            out=rc(xb),
                            in_=rc(src_dram[:, blk * TB : (blk + 1) * TB]
                                   .rearrange("(c p) t -> p c t", p=128)),
                        )
                        for dt in range(2):
                            kacc = ppp.tile([128, TB], f32, tag="pp",
                                            name="kacc")
                            for c in range(8):
                                nc.tensor.matmul(
                                    kacc[:, :],
                                    mm(wkg[:, c, dt * 128 : (dt + 1) * 128]),
                                    mm(xb[:, c, :]),
                                    start=(c == 0),
                                    stop=(c == 7),
                                )
                            nc.vector.tensor_scalar_add(
                                rc(kt[:, dt, blk * TB : (blk + 1) * TB]),
                                kacc[:, :],
                                bk_sb[:, g * 2 + dt : g * 2 + dt + 1],
                            )
                        for tt in range(TB // 128):
                            vacc = ppp.tile([128, GD], f32, tag="pp",
                                            name="vacc")
                            for c in range(8):
                                nc.tensor.matmul(
                                    vacc[:, :],
                                    mm(xb[:, c, tt * 128 : (tt + 1) * 128]),
                                    mm(wvg[:, c, :]),
                                    start=(c == 0),
                                    stop=(c == 7),
                                )
                            j = blk * (TB // 128) + tt
                            dst = vp[:, j, 1 : 1 + 65 * GH].rearrange(
                                "p (h u) -> p h u", u=65
                            )[:, :, 0:HD]
                            nc.vector.tensor_tensor(
                                out=rc(dst),
                                in0=vacc.rearrange("p (h u) -> p h u", u=HD),
                                in1=bv_sb[:, g * GD : (g + 1) * GD].rearrange(
                                    "p (h u) -> p h u", u=HD
                                ),
                                op=Alu.add,
                            )
                    # -- attention for the 4 heads of this group --
                    for h in range(GH):
                        ha = g * GH + h
                        par = h % 2
                        dt = h // 2
                        pc = ppc.tile([128, QTOK], f32, tag="pc", name="pc")
                        vlo = h * 65 + 1   # [v0..v63, one] -> denom row 64
                        for jg in range(8):
                            ps = pps.tile([128, 2, QTOK], f32, tag="ps",
                                          name="ps")
                            for js in range(2):
                                j = jg * 2 + js
                                nc.tensor.matmul(
                                    ps[:, js, :],
                                    mm(kt[par * 64 : par * 64 + 64, dt,
                                          j * 128 : (j + 1) * 128]),
                                    mm(qt[par * 64 : par * 64 + 64, dt, :]),
                                    start=True,
                                    stop=True,
                                )
                            et = pexp.tile([128, 2, QTOK], f32, tag="e",
                                           name="et")
                            nc.scalar.activation(
                                out=rc(et), in_=ps, func=AF.Exp,
                                scale=1.0 / (HD ** 0.5),
                            )
                            for js in range(2):
                                j = jg * 2 + js
                                nc.tensor.matmul(
                                    pc[0:65, :],
                                    mm(vp[:, j, vlo : vlo + 65]),
                                    mm(et[:, js, :]),
                                    start=(jg == 0 and js == 0),
                                    stop=(jg == 7 and js == 1),
                                )
                        # normalize by the denominator row (psum row 64).
                        # the reciprocal broadcast runs as a partition-step-0
                        # DMA so it stays OFF the in-order PE stream: the PE
                        # flows from the last ctx matmul straight into the
                        # next head/group instead of stalling on a bcast mm.
                        rd = psq.tile([128, QTOK], f32, tag="sq", name="rd")
                        nc.vector.reciprocal(
                            out=rd[64:65, :], in_=pc[64:65, :]
                        )
                        # hw partition_broadcast reads physical partition 0:
                        # DMA-shift the value down first (off the PE stream)
                        nc.gpsimd.dma_start(out=rd[0:1, :], in_=rd[64:65, :])
                        rbs = pexp.tile([128, QTOK], f32, tag="rbs",
                                        name="rbs")
                        nc.gpsimd.partition_broadcast(
                            out_ap=rbs[0:64, :], in_ap=rd[0:64, :],
                            channels=64,
                        )
                        cu = pexp.tile([128, QTOK], f32, tag="cu", name="cu")
                        nc.vector.tensor_copy(out=cu[0:64, :], in_=pc[0:64, :])
                        if par == 0:
                            nc.vector.tensor_tensor(
                                out=ctx_tile[0:64, ha // 2, :],
                                in0=cu[0:64, :],
                                in1=rbs[0:64, :],
                                op=Alu.mult,
                            )
                        else:
                            # engines can't cross partitions: normalize at
                            # base 0, then DMA-shift into partitions 64+
                            cn = psq.tile([128, QTOK], f32, tag="sq",
                                          name="cn")
                            nc.vector.tensor_tensor(
                                out=cn[0:64, :],
                                in0=cu[0:64, :],
                                in1=rbs[0:64, :],
                                op=Alu.mult,
                            )
                            nc.gpsimd.dma_start(
                                out=ctx_tile[64:128, ha // 2, :],
                                in_=cn[0:64, :],
                            )

            def layernorm(x1, gname, out_tile, dma_out=None):
                """out = LN(x1) * g + b, normalizing over the partition dim."""
                g_sb, b_sb = sb[f"{gname}_g"], sb[f"{gname}_b"]
                sum_ps = ppp.tile([1, QTOK], f32, tag="pp")
                sq_ps = ppp.tile([1, QTOK], f32, tag="pp")
                for i in range(8):
                    nc.tensor.matmul(
                        sum_ps[:, :], mm(ones[:, 0:1]), mm(x1[:, i, :]),
                        start=(i == 0), stop=(i == 7),
                    )
                for i in range(8):
                    sqt = psq.tile([128, QTOK], f32, tag="sq")
                    nc.vector.tensor_mul(rc(sqt), x1[:, i, :], x1[:, i, :])
                    nc.tensor.matmul(
                        sq_ps[:, :], mm(ones[:, 0:1]), mm(sqt[:, :]),
                        start=(i == 0), stop=(i == 7),
                    )
                s_sb = pstat.tile([128, 2, QTOK], f32, tag="s_sb")
                nc.vector.tensor_copy(out=s_sb[0:1, 0, :], in_=sum_ps[:, :])
                nc.vector.tensor_copy(out=s_sb[0:1, 1, :], in_=sq_ps[:, :])
                meanb = pstat.tile([128, QTOK], f32, tag="meanb")
                nc.gpsimd.partition_broadcast(
                    out_ap=meanb[:, :], in_ap=s_sb[:, 0, :], channels=128
                )
                nc.vector.tensor_scalar_mul(meanb, meanb, 1.0 / D)
                varb = pstat.tile([128, QTOK], f32, tag="varb")
                nc.gpsimd.partition_broadcast(
                    out_ap=varb[:, :], in_ap=s_sb[:, 1, :], channels=128
                )
                nc.vector.tensor_scalar_mul(varb, varb, 1.0 / D)
                tmpb = pstat.tile([128, QTOK], f32, tag="tmpb")
                nc.vector.tensor_mul(tmpb, meanb, meanb)
                nc.vector.tensor_sub(varb, varb, tmpb)
                nc.vector.tensor_scalar_add(varb, varb, EPS)
                nc.vector.reciprocal(out=tmpb, in_=varb)
                rstdb = pstat.tile([128, QTOK], f32, tag="rstdb")
                nc.scalar.activation(out=rstdb, in_=tmpb, func=AF.Sqrt)
                for i in range(8):
                    t1 = psq.tile([128, QTOK], f32, tag="sq")
                    nc.vector.tensor_sub(t1, x1[:, i, :], meanb)
                    nc.vector.tensor_mul(t1, t1, rstdb)
                    nc.vector.tensor_scalar(
                        out=rc(out_tile[:, i, :]), in0=t1,
                        scalar1=g_sb[:, i : i + 1], scalar2=b_sb[:, i : i + 1],
                        op0=Alu.mult, op1=Alu.add,
                    )
                    if dma_out is not None:
                        nc.sync.dma_start(
                            out=dma_out[:, i, :], in_=out_tile[:, i, :]
                        )

            def eight_psums():
                """8 one-bank [128,512] accumulators spanning all three pools."""
                a = [ppp.tile([128, QTOK], f32, tag="pp", name=f"fa{_i}") for _i in range(2)]
                b = [ppc.tile([128, QTOK], f32, tag="pc", name=f"fb{_i}") for _i in range(2)]
                c_ = [pps.tile([128, 2, QTOK], f32, tag="ps", name=f"fc{_i}") for _i in range(2)]
                return [a[0][:, :], a[1][:, :], b[0][:, :], b[1][:, :],
                        c_[0][:, 0, :], c_[0][:, 1, :], c_[1][:, 0, :], c_[1][:, 1, :]]

            # ================= self-attention =================
            ctx1 = pctx.tile([128, 8, QTOK], f32, tag="ctx")
            attention("sa", xT, xq_sb, ctx1)
            x1 = pxa.tile([128, 8, QTOK], f32, tag="xa")
            nc.vector.tensor_add(rc(x1), xq_sb, ctx1)
            x2 = pxa.tile([128, 8, QTOK], f32, tag="xa")
            layernorm(x1, "ln1", x2)

            # ================= cross-attention =================
            ctx2 = pctx.tile([128, 8, QTOK], f32, tag="ctx")
            attention("ca", encT, x2, ctx2)
            x3p = pxa.tile([128, 8, QTOK], f32, tag="xa")
            nc.vector.tensor_add(rc(x3p), x2, ctx2)
            x3 = pqt.tile([128, 8, QTOK], f32, tag="qt")
            layernorm(x3p, "ln2", x3)

            # ================= feed-forward =================
            h2acc = pctx.tile([128, 8, QTOK], f32, tag="ctx")
            for qtr in range(4):
                h1q = pkt.tile([128, 8, QTOK], f32, tag="kt")
                h1accs = eight_psums()
                for c in range(8):
                    f1s = pwslab.tile([128, 1024], f32, tag="wslab")
                    nc.sync.dma_start(
                        out=rc(f1s),
                        in_=rc(fc1_w[c * 128 : (c + 1) * 128,
                                     qtr * 1024 : (qtr + 1) * 1024]),
                    )
                    for f in range(8):
                        nc.tensor.matmul(
                            h1accs[f],
                            mm(f1s[:, f * 128 : (f + 1) * 128]),
                            mm(x3[:, c, :]),
                            start=(c == 0),
                            stop=(c == 7),
                        )
                for f in range(8):
                    nc.scalar.activation(
                        out=rc(h1q[:, f, :]), in_=h1accs[f], func=AF.Relu,
                        bias=fc1b_sb[:, qtr * 8 + f : qtr * 8 + f + 1],
                    )
                h2accs = eight_psums()
                for f in range(8):
                    f2s = pwslab.tile([128, 1024], f32, tag="wslab")
                    nc.sync.dma_start(
                        out=rc(f2s),
                        in_=rc(fc2_w[qtr * 1024 + f * 128
                                     : qtr * 1024 + (f + 1) * 128, :]),
                    )
                    for i in range(8):
                        nc.tensor.matmul(
                            h2accs[i],
                            mm(f2s[:, i * 128 : (i + 1) * 128]),
                            mm(h1q[:, f, :]),
                            start=(f == 0),
                            stop=(f == 7),
                        )
                for i in range(8):
                    if qtr == 0:
                        nc.vector.tensor_copy(out=h2acc[:, i, :], in_=h2accs[i])
                    else:
                        nc.vector.tensor_tensor(
                            out=h2acc[:, i, :], in0=h2acc[:, i, :],
                            in1=h2accs[i], op=Alu.add,
                        )
            x4p = pxa.tile([128, 8, QTOK], f32, tag="xa")
            for i in range(8):
                nc.vector.tensor_scalar_add(
                    h2acc[:, i, :], h2acc[:, i, :], fc2b_sb[:, i : i + 1]
                )
            nc.vector.tensor_add(rc(x4p), x3, h2acc)
            x4 = pxa.tile([128, 8, QTOK], f32, tag="xa")
            layernorm(x4p, "ln3", x4,
                      dma_out=outT.rearrange("(i p) t -> p i t", p=128))

    nc.compile()
    return nc


def _get_program(mode=DTYPE_MODE):
    if mode not in _PROGRAM_CACHE:
        _PROGRAM_CACHE[mode] = _build_program(mode)
    return _PROGRAM_CACHE[mode]


def _make_in_maps(inputs):
    f = np.float32

    def colmajor8(v):  # [1024] -> [128, 8] with [p, i] = v[i*128+p]
        return np.ascontiguousarray(v.reshape(8, 128).T.astype(f))

    shared = {}
    for p in ("sa", "ca"):
        for wn in ("wq", "wk", "wv"):
            shared[f"{p}_{wn}"] = np.ascontiguousarray(inputs[f"{p}_{wn}"], dtype=f)
        for bn in ("bq", "bk"):
            shared[f"{p}_{bn}"] = colmajor8(np.asarray(inputs[f"{p}_{bn}"]))
        shared[f"{p}_bv"] = np.ascontiguousarray(
            np.broadcast_to(np.asarray(inputs[f"{p}_bv"], dtype=f), (128, D))
        )
    shared["fc1_w"] = np.ascontiguousarray(inputs["fc1_w"], dtype=f)
    shared["fc2_w"] = np.ascontiguousarray(inputs["fc2_w"], dtype=f)
    shared["fc1_b"] = np.ascontiguousarray(
        np.asarray(inputs["fc1_b"]).reshape(32, 128).T.astype(f)
    )
    shared["fc2_b"] = colmajor8(np.asarray(inputs["fc2_b"]))
    for i in (1, 2, 3):
        shared[f"ln{i}_g"] = colmajor8(np.asarray(inputs[f"ln{i}_g"]))
        shared[f"ln{i}_b"] = colmajor8(np.asarray(inputs[f"ln{i}_b"]))

    hs = np.asarray(inputs["hidden_states"], dtype=f)
    enc = np.asarray(inputs["encoder_hidden_states"], dtype=f)
    in_maps = []
    for c in range(NCORES):
        b, q0 = c // 4, (c % 4) * QTOK
        m = dict(shared)
        m["xT"] = np.ascontiguousarray(hs[b].T)
        m["xqT"] = np.ascontiguousarray(hs[b, q0 : q0 + QTOK, :].T)
        m["encT"] = np.ascontiguousarray(enc[b].T)
        in_maps.append(m)
    return in_maps


def kernel(**inputs):
    from concourse.bass_utils import run_bass_kernel_spmd

    nc = _get_program()
    in_maps = _make_in_maps(inputs)
    res = run_bass_kernel_spmd(nc, in_maps, core_ids=list(range(NCORES)))
    out = np.empty((2, S, D), np.float32)
    for c in range(NCORES):
        b, q0 = c // 4, (c % 4) * QTOK
        out[b, q0 : q0 + QTOK, :] = res.results[c]["outT"].T
    return out
